# revision 16
# baseline (speedup 1.0000x reference)
"""GRU action encoder on 8 trn2 NeuronCores.

Strategy:
  - Data-parallel: batch N=256 sharded 8 ways (32 per core), weights replicated.
  - Algebraic refactor: with h_enc = h@WencT + c1 and gh = h_enc@WhhT + b_hh,
    fold gh = h@W_big + (a@W_small + b_enc@WhhT + b_hh) where W_big = WencT@WhhT
    and W_small = WencaT@WhhT are precomputed on host (fp64).  The two per-step
    GEMMs (enc: h@WencT, big: h@W_big) then both depend only on h_{t-1} and run
    as one uninterrupted PE stream with no mid-step serialization.
  - Phase 1 (time-parallel, bf16): pre[t] = 40 feature-chunks per step, fp16:
      ch 0-15  e_rz = x@W_ihT[rz] + a@W_small[rz] + (b_ih+b_hh+b_enc@WhhT)[rz]
               (r/z interleaved by gate-group via COLMAP)
      ch 16-23 gn   = x@W_ihT[n] + b_ih[n]
      ch 24-31 dn   = a@W_small[n] + (b_hh + b_enc@WhhT)[n]
      ch 32-39 c1   = a@WencaT + b_enc
  - Scan (bf16 streams, single-pass matmuls = 4x fewer PE cycles than fp32):
    per step 64 weight-streaming matmuls (4 col-strips via tile_position,
    strip j owns K-chunks {j, j+4} so the ci=0 half only reads h chunks 0-3),
    PSUM evacuated to fp16 psb on DVE/ACT, 32 merge matmuls (fp16 R4 selector
    transposes batch-major partials to feature-major + strip-sum), gates in 2
    chunk-groups split across DVE (PSUM adds) / ACT (sigmoid, tanh) / GPSIMD
    (SBUF-only chain) so the next step's first stream pass starts early.
  - 16 steps per For_i iteration to amortize the loop-boundary rendezvous.
"""

import sys

sys.path.insert(0, "/opt/trn_rl_repo")

from contextlib import ExitStack

import ml_dtypes
import numpy as np

import concourse.bacc as bacc
import concourse.mybir as mybir
import concourse.tile as tile
from concourse.bass import ds
from concourse.bass_utils import run_bass_kernel_spmd
from concourse.masks import make_identity

N, T, H, A = 256, 128, 1024, 16
NCORES = 8
NL = N // NCORES  # 32 batch per core
HC = H // 128  # 8 feature chunks
F32 = mybir.dt.float32
F32R = mybir.dt.float32r
FP16 = mybir.dt.float16
BF16 = mybir.dt.bfloat16
AF = mybir.ActivationFunctionType

TB = 16  # timesteps per phase-1 block
NB = T // TB  # 8 blocks
CH = 40  # pre channels per step
# pre channel -> weight column chunk for the 16 e_rz channels:
# ch 0-3 = r chunks 0-3, 4-7 = z chunks 0-3, 8-11 = r 4-7, 12-15 = z 4-7
COLMAP = [0, 1, 2, 3, 8, 9, 10, 11, 4, 5, 6, 7, 12, 13, 14, 15]


def build_program(repeat: int = 1):
    nc = bacc.Bacc("TRN2", target_bir_lowering=False, debug=False, num_devices=NCORES)

    xT = nc.declare_dram_parameter("xT", [H, T, NL], BF16, isOutput=False)
    aT = nc.declare_dram_parameter("aT", [A, T, NL], BF16, isOutput=False)
    WihT = nc.declare_dram_parameter("WihT", [H, 3 * H], BF16, isOutput=False)
    Wsmall = nc.declare_dram_parameter("Wsmall", [A, 3 * H], BF16, isOutput=False)
    WencaT = nc.declare_dram_parameter("WencaT", [A, H], BF16, isOutput=False)
    WencT = nc.declare_dram_parameter("WencT", [H, H], BF16, isOutput=False)
    Wbig = nc.declare_dram_parameter("Wbig", [H, 3 * H], BF16, isOutput=False)
    bias40 = nc.declare_dram_parameter("bias40", [128, CH], F32, isOutput=False)
    R4 = nc.declare_dram_parameter("R4", [128, NL], FP16, isOutput=False)
    h_out = nc.declare_dram_parameter("h_out", [NL, H], F32, isOutput=True)

    # precomputed per-step data, laid out [p][ch][t][n]
    pre_d = nc.dram_tensor("pre_d", [128, CH, T + 4, NL], FP16)
    pre_ap = pre_d.ap()

    with tile.TileContext(nc) as tc:
        with ExitStack() as ctx:
            constp = ctx.enter_context(tc.tile_pool(name="const", bufs=1))
            r4_sb = constp.tile([128, NL], FP16)
            nc.sync.dma_start(r4_sb[:], R4[:])
            bias_sb = constp.tile([128, CH], F32)
            nc.sync.dma_start(bias_sb[:], bias40[:])
            ident = constp.tile([128, 128], F32)
            make_identity(nc, ident[:])

            if repeat > 1:
                rep_cm = tc.For_i(0, repeat, 1)
                rep_cm.__enter__()

            # ---------------- Phase 1: precompute ----------------
            with ExitStack() as pctx:
                pw = pctx.enter_context(tc.tile_pool(name="pre_w", bufs=1))
                wih_sb = pw.tile([128, HC, 3 * H], BF16)
                nc.sync.dma_start(wih_sb[:], WihT.ap().rearrange("(c p) o -> p c o", p=128))
                wsm_sb = pw.tile([A, 3 * H], BF16)
                nc.sync.dma_start(wsm_sb[:], Wsmall[:])
                wea_sb = pw.tile([A, H], BF16)
                nc.sync.dma_start(wea_sb[:], WencaT[:])

                pin = pctx.enter_context(tc.tile_pool(name="pre_in", bufs=2))
                pstg = pctx.enter_context(tc.tile_pool(name="pre_stg", bufs=2))
                pps = pctx.enter_context(tc.tile_pool(name="pre_psum", bufs=4, space="PSUM"))

                xT_r = xT.ap().rearrange("(c p) t n -> p c t n", p=128)
                for tb in range(NB):
                    t0 = tb * TB
                    kx = pin.tile([128, HC, TB, NL], BF16, tag="kx")
                    nc.sync.dma_start(kx[:], xT_r[:, :, t0 : t0 + TB, :])
                    ka = pin.tile([A, TB, NL], BF16, tag="ka")
                    nc.sync.dma_start(ka[:], aT.ap()[:, t0 : t0 + TB, :])

                    stg = None
                    for ch in range(CH):
                        if ch % 10 == 0:
                            stg = pstg.tile([128, 10, TB, NL], FP16, tag="stg")
                        pm = pps.tile([128, TB, NL], F32, tag="pm")
                        if ch < 24:
                            # gi chunk (+ a-fold for rz channels); COLMAP
                            # interleaves e_r/e_z by gate-group
                            m = COLMAP[ch] if ch < 16 else ch
                            for c in range(HC):
                                nc.tensor.matmul(
                                    pm[:],
                                    wih_sb[:, c, 128 * m : 128 * (m + 1)],
                                    kx[:, c],
                                    start=(c == 0),
                                    stop=(c == HC - 1) and ch >= 16,
                                )
                            if ch < 16:
                                nc.tensor.matmul(
                                    pm[:],
                                    wsm_sb[:, 128 * m : 128 * (m + 1)],
                                    ka[:],
                                    start=False,
                                    stop=True,
                                )
                        elif ch < 32:
                            k = ch - 24  # dn chunk: W_small n-columns
                            nc.tensor.matmul(
                                pm[:],
                                wsm_sb[:, 2048 + 128 * k : 2048 + 128 * (k + 1)],
                                ka[:],
                                start=True,
                                stop=True,
                            )
                        else:
                            k = ch - 32  # c1 chunk
                            nc.tensor.matmul(
                                pm[:],
                                wea_sb[:, 128 * k : 128 * (k + 1)],
                                ka[:],
                                start=True,
                                stop=True,
                            )
                        nc.scalar.activation(
                            stg[:, ch % 10], pm[:], AF.Identity,
                            bias=bias_sb[:, ch : ch + 1],
                        )
                        if ch % 10 == 9:
                            nc.sync.dma_start(
                                pre_ap[:, ch - 9 : ch + 1, t0 : t0 + TB, :], stg[:]
                            )

            # ---------------- Phase 2: recurrent scan ----------------
            with ExitStack() as lctx:
                lw = lctx.enter_context(tc.tile_pool(name="loop_w", bufs=1))
                wenc_sb = lw.tile([128, HC, H], BF16)
                nc.sync.dma_start(wenc_sb[:], WencT.ap().rearrange("(c p) o -> p c o", p=128))
                wbig_sb = lw.tile([128, HC, 3 * H], BF16)
                nc.sync.dma_start(wbig_sb[:], Wbig.ap().rearrange("(c p) o -> p c o", p=128))

                state = lctx.enter_context(tc.tile_pool(name="state", bufs=1))
                h_sb = state.tile([128, HC, NL], BF16)
                nc.gpsimd.memset(h_sb[:], 0.0)
                UNROLL = 16
                pres = [
                    state.tile([128, CH, 1, NL], FP16, tag=f"pre{i}", name=f"pre{i}")
                    for i in range(UNROLL)
                ]
                nc.sync.dma_start(pres[0][:], pre_ap[:, :, 0:1, :])

                work = lctx.enter_context(tc.tile_pool(name="work", bufs=3))
                psbp = lctx.enter_context(tc.tile_pool(name="psb", bufs=3))
                lps = lctx.enter_context(tc.tile_pool(name="loop_psum", bufs=1, space="PSUM"))

                # psum bank layout (merge-priority order): R=psA 0-1,
                # N=psA 2-3, E=psA 4-5, Z=psB 6-7.  R streams/stops first so
                # its cast+merges begin earliest (sigmoid r gates the n-path).
                BANKS = [  # (bank, weight tile key, col offset) in stream order
                    (0, "b", 0), (1, "b", 512),        # R
                    (2, "b", 2048), (3, "b", 2560),    # N
                    (4, "e", 0), (5, "e", 512),        # E
                    (6, "b", 1024), (7, "b", 1536),    # Z
                ]
                QSLOT = {"R": 0, "N": 2, "E": 4, "Z": 6}

                def step(pt):
                    # --- stream GEMMs.  Strip j owns K-chunks {j, j+4}: the
                    # ci=0 pass reads only h chunks 0-3 (gate group 0), ci=1
                    # only chunks 4-7.  Strips interleave (j middle) for
                    # col-tile concurrency; bank pairs per (ci,j) visit halve
                    # the h LDWEIGHTS count.
                    psA = lps.tile([128, 6, 512], F32, tag="psA")
                    psB = lps.tile([128, 2, 512], F32, tag="psB")

                    def bank_ap(b):
                        return psA[:, b, :] if b < 6 else psB[:, b - 6, :]

                    for ci in range(2):
                        for sp in range(4):
                            for j in range(4):
                                c = j + 4 * ci
                                for b, wkey, off in BANKS[2 * sp : 2 * sp + 2]:
                                    w = wenc_sb if wkey == "e" else wbig_sb
                                    nc.tensor.matmul(
                                        bank_ap(b)[32 * j : 32 * (j + 1), :],
                                        h_sb[:, c, :],
                                        w[:, c, off : off + 512],
                                        start=(ci == 0), stop=(ci == 1),
                                        tile_position=(0, 32 * j),
                                    )
                    # evacuate to psb fp16 in chain-priority order R, N, E, Z
                    psb = psbp.tile([128, 8, 512], FP16, tag="psb")
                    nc.vector.tensor_copy(psb[:, 0:2, :], psA[:, 0:2, :])
                    nc.scalar.copy(psb[:, 2:4, :], psA[:, 2:4, :])
                    nc.vector.tensor_copy(psb[:, 4:6, :], psA[:, 4:6, :])
                    nc.scalar.copy(psb[:, 6:8, :], psB[:])

                    # --- merges: transpose+strip-sum psb[q, chunk 4g+fi] into
                    # mo slot g*16 + qi*4 + fi.  mo overlays psA banks 0-1 (R,
                    # casted first); group g sits in bank g so group-1 merges
                    # overlap group-0 gate reads.
                    mo = psA[:, 0:2, :].rearrange("p b (q n) -> p (b q) n", n=NL)
                    QORD = ("R", "N", "Z", "E")
                    for g in range(2):
                        for qi, qn in enumerate(QORD):
                            for fi in range(4):
                                col = QSLOT[qn] * 512 + (4 * g + fi) * 128
                                nc.tensor.matmul(
                                    mo[:, g * 16 + qi * 4 + fi, :],
                                    psb[:, col // 512, col % 512 : col % 512 + 128],
                                    r4_sb[:],
                                    start=True, stop=True,
                                )
                        # --- gates for chunk group g (chunks 4g..4g+3);
                        # DVE: PSUM adds, ACT: activations, GPSIMD: SBUF chain.
                        def mog(qi):
                            return mo[:, g * 16 + qi * 4 : g * 16 + qi * 4 + 4, :]

                        def ptc(ch0):
                            return pt[:, ch0 : ch0 + 4, 0, :]

                        rp = work.tile([128, 4, NL], F32, tag="rp")
                        nc.vector.tensor_add(rp[:], mog(0), ptc(8 * g))
                        r = work.tile([128, 4, NL], F32, tag="r")
                        nc.scalar.activation(r[:], rp[:], AF.Sigmoid)
                        t1 = work.tile([128, 4, NL], F32, tag="t1")
                        nc.vector.tensor_add(t1[:], mog(1), ptc(24 + 4 * g))
                        t2 = work.tile([128, 4, NL], F32, tag="t2")
                        nc.gpsimd.tensor_mul(t2[:], r[:], t1[:])
                        t3 = work.tile([128, 4, NL], F32, tag="t3")
                        nc.gpsimd.tensor_add(t3[:], t2[:], ptc(16 + 4 * g))
                        ngate = work.tile([128, 4, NL], F32, tag="ngate")
                        nc.scalar.activation(ngate[:], t3[:], AF.Tanh)
                        zp = work.tile([128, 4, NL], F32, tag="zp")
                        nc.vector.tensor_add(zp[:], mog(2), ptc(8 * g + 4))
                        z = work.tile([128, 4, NL], F32, tag="z")
                        nc.scalar.activation(z[:], zp[:], AF.Sigmoid)
                        he = work.tile([128, 4, NL], F32, tag="he")
                        nc.vector.tensor_add(he[:], mog(3), ptc(32 + 4 * g))
                        d = work.tile([128, 4, NL], F32, tag="d")
                        nc.gpsimd.tensor_sub(d[:], he[:], ngate[:])
                        zd = work.tile([128, 4, NL], F32, tag="zd")
                        nc.gpsimd.tensor_mul(zd[:], z[:], d[:])
                        nc.gpsimd.tensor_add(h_sb[:, 4 * g : 4 * g + 4, :], ngate[:], zd[:])

                with tc.For_i(0, T, UNROLL, hint_engines=(mybir.EngineType.PE,)) as iv:
                    for k in range(UNROLL):
                        nxt = pres[(k + 1) % UNROLL]
                        nc.sync.dma_start(
                            nxt[:], pre_ap[:, :, ds(iv + k + 1, 1), :]
                        )
                        step(pres[k])

                # ---------------- Phase 3: output ----------------
                # outP overlays psA banks 2-3 (PSUM is fully allocated already)
                h32 = work.tile([128, HC, NL], F32, tag="h32")
                nc.vector.tensor_copy(h32[:], h_sb[:])
                psA_f = lps.tile([128, 6, 512], F32, tag="psA")
                outP = psA_f[0:32, 2:4, :].rearrange("p b (c o) -> p (b c) o", o=128)
                for c in range(HC):
                    nc.tensor.transpose(outP[:, c, :], h32[:, c, :], ident[:])
                hout = work.tile([NL, HC, 128], F32, tag="hout")
                nc.vector.tensor_copy(hout[:], outP[:])
                nc.sync.dma_start(h_out.ap().rearrange("n (c o) -> n c o", c=HC), hout[:])

            if repeat > 1:
                rep_cm.__exit__(None, None, None)

    nc.compile()
    return nc


_cache = {}


def _get_program(repeat: int = 1):
    if repeat not in _cache:
        _cache[repeat] = build_program(repeat)
    return _cache[repeat]


def _prep_inputs(embedding, actions, W_enc, b_enc, W_ih, W_hh, b_ih, b_hh):
    f = np.float32
    xT = np.ascontiguousarray(np.asarray(embedding, f).transpose(2, 1, 0))  # [H,T,N]
    aT = np.ascontiguousarray(np.asarray(actions, f).transpose(2, 1, 0))  # [A,T,N]
    W_enc = np.asarray(W_enc, np.float64)
    W_hh = np.asarray(W_hh, np.float64)
    WencT = W_enc[:, :H].T  # [H, H]
    WencaT = W_enc[:, H:].T  # [A, H]
    WhhT = W_hh.T  # [H, 3H]
    Wbig = WencT @ WhhT  # [H, 3H]
    Wsmall = WencaT @ WhhT  # [A, 3H]
    b_ih = np.asarray(b_ih, np.float64)
    b_hh = np.asarray(b_hh, np.float64)
    b_enc = np.asarray(b_enc, np.float64)
    gbias = b_enc @ WhhT + b_hh  # [3H]
    e_bias = b_ih[: 2 * H] + gbias[: 2 * H]  # rz channels
    gn_bias = b_ih[2 * H :]
    dn_bias = gbias[2 * H :]
    # bias40 [128, CH]: value for (partition p, channel ch)
    bias40 = np.empty((128, CH), f)
    for k in range(16):
        m = COLMAP[k]
        bias40[:, k] = e_bias[128 * m : 128 * (m + 1)]
    for k in range(8):
        bias40[:, 16 + k] = gn_bias[128 * k : 128 * (k + 1)]
        bias40[:, 24 + k] = dn_bias[128 * k : 128 * (k + 1)]
        bias40[:, 32 + k] = b_enc[128 * k : 128 * (k + 1)]
    R4 = np.zeros((128, NL), np.float16)
    R4[np.arange(128), np.arange(128) % NL] = 1.0
    common = dict(
        WihT=np.ascontiguousarray(np.asarray(W_ih, f).T).astype(ml_dtypes.bfloat16),
        Wsmall=Wsmall.astype(f).astype(ml_dtypes.bfloat16),
        WencaT=np.ascontiguousarray(WencaT.astype(f)).astype(ml_dtypes.bfloat16),
        WencT=np.ascontiguousarray(WencT.astype(np.float32)).astype(ml_dtypes.bfloat16),
        Wbig=Wbig.astype(np.float32).astype(ml_dtypes.bfloat16),
        bias40=bias40,
        R4=R4,
    )
    in_maps = []
    for k in range(NCORES):
        m = dict(common)
        m["xT"] = np.ascontiguousarray(xT[:, :, k * NL : (k + 1) * NL]).astype(ml_dtypes.bfloat16)
        m["aT"] = np.ascontiguousarray(aT[:, :, k * NL : (k + 1) * NL]).astype(ml_dtypes.bfloat16)
        in_maps.append(m)
    return in_maps


def run(inputs: dict, repeat: int = 1, trace: bool = False):
    nc = _get_program(repeat)
    in_maps = _prep_inputs(**inputs)
    res = run_bass_kernel_spmd(nc, in_maps, list(range(NCORES)), trace=trace)
    out = np.concatenate([res.results[k]["h_out"] for k in range(NCORES)], axis=0)
    if trace:
        return out, res
    return out


def kernel(**inputs) -> np.ndarray:
    return run(inputs, repeat=1)
